# revision 22
# baseline (speedup 1.0000x reference)
"""Trainium2 Bass kernel for nn_ConstrainedEnhancementModel.

Model: x(512,256,32) -> flatten(512,8192) -> MLP encoder/decoder
(8192->1024->512->256->512->1024->131072) -> constraint blend with
linearly-interpolated low-res data.

Key transformation (host-side fold): the post-matmul constraint
   out = where(knot, x_knot, where(in_seg, 0.8*interp + 0.2*decoded, decoded))
is linear in x, so it folds into the final projection:
 - scale w6/b6 columns by 0 (knot), 0.2 (in-segment) or 1.0 (tail)
 - per 512-column segment, a K=65 bf16 matmul block (start/end x values
   + ones-row carrying b6_eff) initializes the PSUM accumulation.
Device kernel is then a pure matmul pipeline (no gather/select on chip).

Precision plan (rel-err budget 2e-2): the dominant projection runs in
fp8 e4m3 DoubleRow.  The fp8 scale is split sqrt(32)/sqrt(32): w5 is
pre-divided by sqrt(32) on host (so d2 = relu(...)/sqrt32 exactly) and
w6 pre-multiplied by sqrt(32); PSUM accumulates TRUE values and
evacuation is a plain f32->bf16 copy.  Encoder runs bf16.  Output is
bf16, upcast on host; knot columns overwritten host-side in f32.

Sharding over 8 cores (v3): L1 is N-SHARDED — each core computes its
own 128 of the 1024 L1 features over the FULL contraction (full x,
w1[:, 128c:128c+128]), applies relu locally, and a single AllGather
(1MB payload, no reduction) replicates h1.  This replaces the v2
AllReduce (measured ~60us end-to-end mesh) with a ~half-cost gather and
removes the pre-activation round-trip.  L2-L5 replicated; the
1024x131072 projection is column-sharded (core c owns output timesteps
[c*512, (c+1)*512)).

Perf notes (from perfetto): PE moving-operand ingest is ~2B/cycle/
partition, so matmul time ~ N * dtype_bytes (fp8-DR N=512 == bf16
N=512 == ~0.26us); the big phase is a tensor-saturated stream of
128 out-tiles x (1 interp + 4 DR) passes.  One LDWEIGHTS is emitted
per matmul and never elided; it overlaps the previous matmul.  DMAs:
w6 pre-tiled [seg, 128, 8, 512] -> one contiguous 512KB DMA per
segment (4KB partition lines); w2..w5 as one packed blob; rep-startup
loads are emitted in consumption order so L1's weight/x stream is not
stuck behind bulk loads (v2 lost 36us/rep to queue-FIFO inversion).
"""

from contextlib import ExitStack

import numpy as np
from ml_dtypes import bfloat16, float8_e4m3

import concourse.bacc as bacc
import concourse.mybir as mybir
import concourse.tile as tile
from concourse.bass import ds, ts
from concourse.bass_utils import run_bass_kernel_spmd

DT = mybir.dt

B, L, F, H, HID = 512, 256, 32, 4096, 512
UP = H // L          # 16 timesteps per low-res segment
LF = L * F           # 8192
HF = H * F           # 131072
NCORES = 8
COLS = HF // NCORES  # 16384 output cols per core
SEGC = UP * F        # 512 cols per segment
NSEG = COLS // SEGC  # 32 segments per core
KI = 2 * F           # 64: interp-block contraction size (b6 is added host-side)
NBIAS = 1 + 4 + 2 + 4 + 8  # packed bias columns (b1 is the core's slice)
RT32 = float(np.sqrt(32.0))  # fp8 scale split: w5 /= RT32, w6 *= RT32
OG = 4               # segments per output-staging group
# packed w2..w5 blob: per-layer (k_tiles, m_tiles, free-offset)
WCAT = {2: (8, 4, 0), 3: (4, 2, 4096), 4: (2, 4, 5120), 5: (4, 8, 6144)}
WCATF = 10240
BOFF = {1: 0, 2: 1, 3: 5, 4: 7, 5: 11}

_CACHE: dict = {}
_SIM_NOCC = False  # sim-only: replace the collective with local DMA copies


def _build_program(reps=1, phase="all"):
    """One SPMD program; per-core data differences live in the inputs.

    reps>1 repeats the whole body back-to-back inside one NEFF — used only
    by the timing harness (per-exec time = wall-time delta / extra reps).
    phase: "all" | "enc" (encoder only) | "big" (projection only, dummy d2).
    """
    bf16, f32, fp8 = DT.bfloat16, DT.float32, DT.float8e4
    DR = mybir.MatmulPerfMode.DoubleRow
    nc = bacc.Bacc("TRN2", target_bir_lowering=False, debug=False, num_devices=NCORES)

    xTf = nc.dram_tensor("xTf", [LF, B], bf16, kind="ExternalInput")
    w1n = nc.dram_tensor("w1n", [LF, 128], bf16, kind="ExternalInput")
    agin = nc.dram_tensor("agin", [128, B], bf16)
    agout = nc.dram_tensor("agout", [2 * HID, B], bf16, addr_space="Shared")
    wcat = nc.dram_tensor("wcat", [128, WCATF], bf16, kind="ExternalInput")
    bpk = nc.dram_tensor("bpk", [128, NBIAS], f32, kind="ExternalInput")
    w6p = nc.dram_tensor("w6p", [NSEG, 128, 8, SEGC], fp8, kind="ExternalInput")
    xv = nc.dram_tensor("xv", [KI, NSEG, B], bf16, kind="ExternalInput")
    # cf slot 0 = standard coef block; slot 1 = per-core data for the LAST
    # local segment (the 'last' block on core 7, std elsewhere) — keeps the
    # SPMD program uniform while the data differs per core.
    cf = nc.dram_tensor("cf", [KI, 2, SEGC], bf16, kind="ExternalInput")
    out = nc.dram_tensor("out", [B, COLS], bf16, kind="ExternalOutput")

    RELU = mybir.ActivationFunctionType.Relu
    IDENT = mybir.ActivationFunctionType.Identity

    with tile.TileContext(nc) as tc, ExitStack() as _pools:
        # pools are shared across reps so the Tile scheduler can pipeline
        # rep N+1's loads/L1 into rep N's big phase (per-rep pools forced a
        # full drain at every rep boundary — ~30us of tensor idle each).
        psum = _pools.enter_context(tc.tile_pool(name="psum", bufs=8, space="PSUM"))
        xpool = _pools.enter_context(tc.tile_pool(name="xpool", bufs=4))
        wpool = _pools.enter_context(tc.tile_pool(name="wpool", bufs=2))
        scratch = _pools.enter_context(tc.tile_pool(name="scratch", bufs=2))
        acts = _pools.enter_context(tc.tile_pool(name="acts", bufs=2))
        cpool = _pools.enter_context(tc.tile_pool(name="cpool", bufs=1))
        w6pool = _pools.enter_context(tc.tile_pool(name="w6pool", bufs=4))
        opool = _pools.enter_context(tc.tile_pool(name="opool", bufs=2))

        def _one_rep(rep):

            def _enc():
                # ---- L1, N-sharded: this core's 128 features, full K ----
                # Emission order = consumption order so the per-queue FIFO
                # keeps the L1 stream fed before the bulk loads land.
                btile = cpool.tile([128, NBIAS], f32, name=f"btile{rep}", tag="btile")
                nc.scalar.dma_start(btile[:], bpk[:])
                w1t = [
                    wpool.tile([128, 32, 128], bf16, name=f"w1t{rep}_{wc}", tag="w")
                    for wc in range(2)
                ]
                nc.sync.dma_start(
                    w1t[0][:],
                    w1n[ds(0, 4096), :].rearrange("(k p) d -> p k d", p=128),
                )
                xts = []
                for xc in range(8):
                    xt = xpool.tile([128, 8, B], bf16, name=f"xt{rep}_{xc}", tag="xt")
                    eng = nc.scalar if xc % 2 == 0 else nc.sync
                    eng.dma_start(
                        xt[:],
                        xTf[ds(xc * 1024, 1024), :].rearrange(
                            "(k p) d -> p k d", p=128
                        ),
                    )
                    xts.append(xt)
                    if xc == 1:
                        nc.sync.dma_start(
                            w1t[1][:],
                            w1n[ds(4096, 4096), :].rearrange(
                                "(k p) d -> p k d", p=128
                            ),
                        )
                # bulk loads for later phases, behind the L1 stream
                xvt = cpool.tile([KI, NSEG, B], bf16, name=f"xvt{rep}", tag="xvt")
                nc.sync.dma_start(xvt[:], xv[:])
                cft = cpool.tile([KI, 2, SEGC], bf16, name=f"cft{rep}", tag="cft")
                nc.scalar.dma_start(cft[:], cf[:])
                wct = cpool.tile([128, WCATF], bf16, name=f"wct{rep}", tag="wct")
                nc.scalar.dma_start(wct[:], wcat[:])

                h1ps = psum.tile([128, B], f32, tag="psum", name=f"h1ps{rep}")
                for kk in range(64):
                    nc.tensor.matmul(
                        h1ps[:],
                        w1t[kk // 32][:, kk % 32, :],
                        xts[kk // 8][:, kk % 8, :],
                        start=(kk == 0),
                        stop=(kk == 63),
                    )
                h1c = scratch.tile([128, B], bf16, tag="s", name=f"h1c{rep}")
                nc.scalar.activation(h1c[:], h1ps[:], RELU, bias=btile[:, 0:1])
                nc.sync.dma_start(agin[:], h1c[:])
                if _SIM_NOCC:
                    for m in range(8):
                        nc.gpsimd.dma_start(agout[ds(m * 128, 128), :], agin[:])
                else:
                    nc.gpsimd.collective_compute(
                        "AllGather",
                        mybir.AluOpType.bypass,
                        replica_groups=[list(range(NCORES))],
                        ins=[agin[:]],
                        outs=[agout[:]],
                    )
                htmp = scratch.tile([128, 8, B], bf16, tag="s", name=f"htmp{rep}")
                nc.sync.dma_start(
                    htmp[:, 0:4, :],
                    agout[ds(0, 512), :].rearrange("(m p) d -> p m d", p=128),
                )
                nc.scalar.dma_start(
                    htmp[:, 4:8, :],
                    agout[ds(512, 512), :].rearrange("(m p) d -> p m d", p=128),
                )

                # ---- L2..L5 from the packed weight blob (zero per-layer DMA) ----
                def mlp_layer(lid, rhs, func, name, pool, out_dtype=bf16):
                    k_tiles, m_tiles, off = WCAT[lid]
                    o = pool.tile(
                        [128, m_tiles, B], out_dtype,
                        tag="s" if pool is scratch else name, name=f"{name}{rep}",
                    )
                    ps = [
                        psum.tile([128, B], f32, tag="psum", name=f"ps{rep}_{name}_{m}")
                        for m in range(m_tiles)
                    ]
                    for kc in range(k_tiles):
                        for m in range(m_tiles):
                            nc.tensor.matmul(
                                ps[m][:],
                                wct[:, ds(off + kc * m_tiles * 128 + m * 128, 128)],
                                rhs[:, kc, :],
                                start=(kc == 0),
                                stop=(kc == k_tiles - 1),
                            )
                    ob = BOFF[lid]
                    for m in range(m_tiles):
                        nc.scalar.activation(
                            o[:, m, :], ps[m][:], func,
                            bias=btile[:, ob + m : ob + m + 1],
                        )
                    return o

                h2 = mlp_layer(2, htmp, RELU, "h2", scratch)
                ft = mlp_layer(3, h2, IDENT, "ft", scratch)
                d1 = mlp_layer(4, ft, RELU, "d1", scratch)
                d2 = mlp_layer(5, d1, RELU, "d2", acts, out_dtype=fp8)
                return d2, xvt, cft

            def _big(d2, xvt, cft):
                # ---- big projection + folded constraint, 32 segments ----
                # Per 512-col segment and batch tile b: one PSUM bank: bf16
                # interp matmul (start, K=65) + 4 fp8 DoubleRow matmuls
                # (K=256 each, N=512 = full bank).
                for g in range(NSEG // OG):
                    ot = [
                        opool.tile(
                            [128, OG, SEGC], bf16, tag=f"ot{b}", name=f"ot{rep}_{g}_{b}"
                        )
                        for b in range(4)
                    ]
                    for si in range(OG):
                        s = g * OG + si
                        w6t = w6pool.tile(
                            [128, 8, SEGC], fp8, name=f"w6t{rep}_{s}", tag="w6"
                        )
                        eng = nc.sync if s % 2 == 0 else nc.scalar
                        eng.dma_start(w6t[:], w6p[s])
                        if s < 6:
                            # HAM keep-alive: a throwaway matmul chained to this
                            # w6 DMA arrival keeps the PE activity window busy
                            # during the AllGather wait, so L2..L5 and the early
                            # projection run at 2.4GHz instead of cold 1.2GHz.
                            jp = psum.tile(
                                [128, SEGC], f32, tag="psum", name=f"ka{rep}_{s}"
                            )
                            nc.tensor.matmul(
                                jp[:], w6t[:, 0, ts(0, 128)], w6t[:, 0, :],
                                start=True, stop=True,
                            )
                        pso = [
                            psum.tile(
                                [128, SEGC], f32, tag="psum", name=f"pso{rep}_{s}_{b}"
                            )
                            for b in range(4)
                        ]
                        ci = 1 if s == NSEG - 1 else 0
                        for b in range(4):
                            nc.tensor.matmul(
                                pso[b][:],
                                xvt[:, s, ts(b, 128)],
                                cft[:, ci, :],
                                start=True,
                                stop=False,
                            )
                        for kp in range(4):
                            for b in range(4):
                                nc.tensor.matmul(
                                    pso[b][:],
                                    d2[:, ds(2 * kp, 2), ts(b, 128)],
                                    w6t[:, ds(2 * kp, 2), :],
                                    start=False,
                                    stop=(kp == 3),
                                    perf_mode=DR,
                                )
                        for b in range(4):
                            nc.vector.tensor_copy(ot[b][:, si, :], pso[b][:])
                    for b in range(4):
                        eng = nc.sync if b % 2 == 0 else nc.scalar
                        eng.dma_start(
                            out[ts(b, 128), ds(g * OG * SEGC, OG * SEGC)].rearrange(
                                "p (a x) -> p a x", a=OG
                            ),
                            ot[b][:],
                        )

            if phase == "enc":
                d2, xvt, cft = _enc()
                otx = opool.tile([128, B], bf16, name=f"otx{rep}", tag="ot0")
                nc.vector.tensor_copy(otx[:], d2[:, 0, :])
                nc.sync.dma_start(out[ts(0, 128), ts(0, B)], otx[:])
            elif phase == "big":
                d2 = acts.tile([128, 8, B], fp8, tag="d2", name=f"d2x{rep}")
                nc.vector.memset(d2[:].bitcast(DT.uint32), 1)
                xvt = cpool.tile([KI, NSEG, B], bf16, name=f"xvt{rep}", tag="xvt")
                nc.sync.dma_start(xvt[:], xv[:])
                cft = cpool.tile([KI, 2, SEGC], bf16, name=f"cft{rep}", tag="cft")
                nc.scalar.dma_start(cft[:], cf[:])
                _big(d2, xvt, cft)
            else:
                d2, xvt, cft = _enc()
                _big(d2, xvt, cft)

        for _rep in range(reps):
            _one_rep(_rep)

    nc.compile()
    return nc


def _host_prep(inputs):
    """Shard + fold. Returns per-core input maps."""
    x = np.ascontiguousarray(inputs["low_res_data"], dtype=np.float32)
    x2d = x.reshape(B, LF)
    xTa = np.ascontiguousarray(x2d.T)
    w6 = np.asarray(inputs["w6"], dtype=np.float32)
    b6 = np.asarray(inputs["b6"], dtype=np.float32)

    # per-output-column scale: 0 on knots, 0.2 in-segment, 1.0 in the tail
    h = np.arange(H)
    colscale = np.where(h % UP == 0, 0.0, np.where(h < (L - 1) * UP, 0.2, 1.0))
    colscale = np.repeat(colscale, F).astype(np.float32)  # (HF,)
    b6_eff = b6 * colscale

    # interp coefficient blocks (shared by all segments except the last);
    # b6's contribution is a per-column constant, added host-side in kernel()
    fidx = np.arange(F)
    std = np.zeros((KI, SEGC), np.float32)
    last = np.zeros((KI, SEGC), np.float32)
    for h_off in range(UP):
        a = h_off / UP
        cs = 1.0 if h_off == 0 else 0.8 * (1.0 - a)
        ce = 0.0 if h_off == 0 else 0.8 * a
        std[fidx, h_off * F + fidx] = cs
        last[fidx, h_off * F + fidx] = 1.0 if h_off == 0 else 0.0
        std[F + fidx, h_off * F + fidx] = ce

    # packed w2..w5 blob [128, WCATF]: wcat[p, off+kc*m128+j] = wL[kc*128+p, j]
    wcat = np.zeros((128, WCATF), np.float32)
    for lid, scale in ((2, 1.0), (3, 1.0), (4, 1.0), (5, 1.0 / RT32)):
        k_tiles, m_tiles, off = WCAT[lid]
        wl = np.asarray(inputs[f"w{lid}"], np.float32) * scale  # [k*128, m*128]
        blk = wl.reshape(k_tiles, 128, m_tiles * 128).transpose(1, 0, 2)
        wcat[:, off : off + k_tiles * m_tiles * 128] = blk.reshape(128, -1)

    w1f = np.asarray(inputs["w1"], np.float32)
    xTb = xTa.astype(bfloat16)
    shared = {"wcat": wcat.astype(bfloat16), "xTf": xTb}

    # biases (b1 column is per-core, filled below)
    bpk_base = np.zeros((128, NBIAS), np.float32)
    off = 1
    for i in (2, 3, 4, 5):
        bv = np.asarray(inputs[f"b{i}"], np.float32)
        if i == 5:
            bv = bv / RT32  # matches host-side w5 /= RT32 (d2 scale split)
        m = bv.shape[0] // 128
        bpk_base[:, off : off + m] = bv.reshape(m, 128).T
        off += m
    b1 = np.asarray(inputs["b1"], np.float32)

    in_maps = []
    for c in range(NCORES):
        j0 = c * COLS
        # pre-tiled w6 [NSEG, 128, 8, SEGC]: one contiguous DMA per segment
        w6c = (
            w6[:, j0 : j0 + COLS] * colscale[j0 : j0 + COLS] * RT32
        ).astype(float8_e4m3)
        w6pc = np.ascontiguousarray(
            w6c.reshape(8, 128, NSEG, SEGC).transpose(2, 1, 0, 3)
        )
        w1sl = np.ascontiguousarray(w1f[:, c * 128 : (c + 1) * 128]).astype(bfloat16)
        bpk = bpk_base.copy()
        bpk[:, 0] = b1[c * 128 : (c + 1) * 128]
        xvc = np.zeros((KI, NSEG, B), np.float32)
        for sl in range(NSEG):
            s = c * NSEG + sl
            xvc[0:F, sl] = xTa[s * F : (s + 1) * F]
            if s + 1 < L:
                xvc[F : 2 * F, sl] = xTa[(s + 1) * F : (s + 2) * F]
        cfc = np.stack(
            [std, last if (c + 1) * NSEG >= L else std], axis=1
        )  # [KI, 2, SEGC]
        in_maps.append(
            {
                **shared,
                "w6p": w6pc,
                "xv": xvc.astype(bfloat16),
                "cf": cfc.astype(bfloat16),
                "w1n": w1sl,
                "bpk": bpk,
            }
        )
    return in_maps


def kernel(**inputs):
    if "nc" not in _CACHE:
        _CACHE["nc"] = _build_program()
    nc = _CACHE["nc"]
    in_maps = _host_prep(inputs)
    res = run_bass_kernel_spmd(nc, in_maps, list(range(NCORES)))
    out = np.empty((B, H, F), np.float32)
    for c in range(NCORES):
        out[:, c * (H // NCORES) : (c + 1) * (H // NCORES), :] = (
            res.results[c]["out"].astype(np.float32).reshape(B, H // NCORES, F)
        )
    # b6's (column-constant) contribution, dropped from the device fold
    b6 = np.asarray(inputs["b6"], np.float32)
    h = np.arange(H)
    colscale = np.where(h % UP == 0, 0.0, np.where(h < (L - 1) * UP, 0.2, 1.0))
    out += (b6 * np.repeat(colscale, F)).reshape(1, H, F)
    # knot timesteps are exact copies of the low-res input; write them in f32
    out[:, ::UP, :] = np.asarray(inputs["low_res_data"], np.float32)
    return out


# revision 34
# speedup vs baseline: 1.0563x; 1.0563x over previous
"""Trainium2 Bass kernel for nn_ConstrainedEnhancementModel.

Model: x(512,256,32) -> flatten(512,8192) -> MLP encoder/decoder
(8192->1024->512->256->512->1024->131072) -> constraint blend with
linearly-interpolated low-res data.

Key transformation (host-side fold): the post-matmul constraint
   out = where(knot, x_knot, where(in_seg, 0.8*interp + 0.2*decoded, decoded))
is linear in x, so it folds into the final projection:
 - scale w6/b6 columns by 0 (knot), 0.2 (in-segment) or 1.0 (tail)
 - per 512-column segment, append a K=65 matmul block:
   32 rows of x (segment start vals), 32 rows (segment end vals), and a
   ones-row carrying the bias b6_eff.
Device kernel is then a pure matmul pipeline (no gather/select on chip).

Precision plan (rel-err budget 2e-2): the dominant projection runs in
fp8 e4m3 with MatmulPerfMode.DoubleRow (K=256/instr, 2x PE rate).  w6 is
pre-scaled by 32 so its N(0, 1/32) entries use e4m3's normal range; the
interp coefficient block is pre-scaled by 32 in bf16, and the PSUM is
descaled by exactly 1/32 during evacuation.  decoded carries only 0.2
weight in most of the output, so its ~4% fp8 error contributes ~1%.
Encoder runs in bf16 end-to-end (including the AllReduce payload).
Output is written bf16 and upcast on host; knot columns are overwritten
host-side with exact f32 input values.

Sharding over 8 cores: encoder L1 is K-sharded + AllReduce at full
batch in transposed layout [features, batch]; L2-L5 replicated; the
1024x131072 projection is tensor-parallel column-sharded (16384 cols =
512 output timesteps per core); core c writes output rows [c*512,
(c+1)*512) of H.

Perf notes: DMA instruction issue costs ~0.6-1us of sequencer time, so
everything is loaded with few, large multi-dim-AP DMAs split across the
two HWDGE-capable engines (sync, scalar).
"""

from contextlib import ExitStack

import numpy as np
from ml_dtypes import bfloat16, float8_e4m3

import concourse.bacc as bacc
import concourse.mybir as mybir
import concourse.tile as tile
from concourse.bass import ds, ts
from concourse.bass_utils import run_bass_kernel_spmd

DT = mybir.dt

B, L, F, H, HID = 512, 256, 32, 4096, 512
UP = H // L          # 16 timesteps per low-res segment
LF = L * F           # 8192
HF = H * F           # 131072
NCORES = 8
COLS = HF // NCORES  # 16384 output cols per core
SEGC = UP * F        # 512 cols per segment
NSEG = COLS // SEGC  # 32 segments per core
KI = 2 * F + 1       # 65: interp-block contraction size
NBIAS = 8 + 4 + 2 + 4 + 8  # packed bias columns
W6SCALE = 32.0       # fp8 pre-scale on w6 / interp coeffs; descaled at PSUM

_CACHE: dict = {}
_SIM_NOCC = False  # sim-only: replace the collective with a local DMA copy
_USE_DR = True     # debug: False = plain fp8 matmuls (K=128) instead of DoubleRow


def _build_program(reps=1, phase="all"):
    """One SPMD program; per-core data differences live in the inputs.

    reps>1 repeats the whole body back-to-back inside one NEFF — used only
    by the timing harness (per-exec time = wall-time delta / extra reps).
    phase: "all" | "enc" (encoder only) | "big" (projection only, dummy d2).
    """
    bf16, f32, fp8 = DT.bfloat16, DT.float32, DT.float8e4
    DR = mybir.MatmulPerfMode.DoubleRow
    nc = bacc.Bacc("TRN2", target_bir_lowering=False, debug=False, num_devices=NCORES)

    KSH = LF // NCORES  # 1024 contraction rows of layer 1 per core
    xTs = nc.dram_tensor("xTs", [KSH, B], bf16, kind="ExternalInput")
    w1s = nc.dram_tensor("w1s", [KSH, 2 * HID], bf16, kind="ExternalInput")
    arin = nc.dram_tensor("arin", [2 * HID, B], bf16)
    arout = nc.dram_tensor("arout", [2 * HID, B], bf16, addr_space="Shared")
    w2 = nc.dram_tensor("w2", [2 * HID, HID], bf16, kind="ExternalInput")
    w3 = nc.dram_tensor("w3", [HID, HID // 2], bf16, kind="ExternalInput")
    w4 = nc.dram_tensor("w4", [HID // 2, HID], bf16, kind="ExternalInput")
    w5 = nc.dram_tensor("w5", [HID, 2 * HID], bf16, kind="ExternalInput")
    bpk = nc.dram_tensor("bpk", [128, NBIAS], f32, kind="ExternalInput")
    w6e = nc.dram_tensor("w6e", [2 * HID, COLS], fp8, kind="ExternalInput")
    ipk = nc.dram_tensor("ipk", [NSEG, KI, 2, SEGC], bf16, kind="ExternalInput")
    out = nc.dram_tensor("out", [B, COLS], bf16, kind="ExternalOutput")

    RELU = mybir.ActivationFunctionType.Relu
    IDENT = mybir.ActivationFunctionType.Identity

    SGRP = 2  # segments per output-staging group

    with tile.TileContext(nc) as tc, ExitStack() as _pools:
        # pools are shared across reps so the Tile scheduler can pipeline rep
        # N+1's loads and L1 into rep N's projection tail (per-rep pools force
        # a full drain at every rep boundary — tens of us of tensor idle).
        psum = _pools.enter_context(tc.tile_pool(name="psum", bufs=8, space="PSUM"))
        xpool = _pools.enter_context(tc.tile_pool(name="xpool", bufs=4))
        wpool = _pools.enter_context(tc.tile_pool(name="wpool", bufs=5))
        scratch = _pools.enter_context(tc.tile_pool(name="scratch", bufs=2))
        acts = _pools.enter_context(tc.tile_pool(name="acts", bufs=2))
        bpool = _pools.enter_context(tc.tile_pool(name="bpool", bufs=1))
        w6pool = _pools.enter_context(tc.tile_pool(name="w6pool", bufs=3))
        ipool = _pools.enter_context(tc.tile_pool(name="ipool", bufs=2))
        opool = _pools.enter_context(tc.tile_pool(name="opool", bufs=2))

        def _one_rep(rep):
            btile = bpool.tile([128, NBIAS], f32, name=f"btile{rep}", tag="btile")
            nc.scalar.dma_start(btile[:], bpk[:])
            boff = {1: 0, 2: 8, 3: 12, 4: 14, 5: 18}

            def _enc():
                # ---- L1: K-sharded partial matmul + AllReduce over 8 cores ----
                ps1 = [
                    psum.tile([128, B], f32, tag="psum", name=f"ps1_{rep}_{m}")
                    for m in range(8)
                ]
                for kc in range(4):
                    e1 = nc.sync if kc % 2 == 0 else nc.scalar
                    e2 = nc.scalar if kc % 2 == 0 else nc.sync
                    xt = xpool.tile([128, 2, B], bf16, name=f"xt{rep}_{kc}", tag="xt")
                    e2.dma_start(
                        xt[:],
                        xTs[ds(kc * 256, 256), :].rearrange("(k p) d -> p k d", p=128),
                    )
                    w1t = wpool.tile(
                        [128, 2, 2 * HID], bf16, name=f"w1t{rep}_{kc}", tag="w"
                    )
                    e1.dma_start(
                        w1t[:],
                        w1s[ds(kc * 256, 256), :].rearrange("(k p) d -> p k d", p=128),
                    )
                    for k4 in range(2):
                        for m in range(8):
                            nc.tensor.matmul(
                                ps1[m][:],
                                w1t[:, k4, ts(m, 128)],
                                xt[:, k4, :],
                                start=(kc == 0 and k4 == 0),
                                stop=(kc == 3 and k4 == 1),
                            )
                hp = scratch.tile([128, 8, B], bf16, tag="s", name=f"hp{rep}")
                for m in range(8):
                    nc.vector.tensor_copy(hp[:, m, :], ps1[m][:])
                nc.sync.dma_start(
                    arin[ds(0, 512), :].rearrange("(m p) d -> p m d", p=128),
                    hp[:, 0:4, :],
                )
                nc.scalar.dma_start(
                    arin[ds(512, 512), :].rearrange("(m p) d -> p m d", p=128),
                    hp[:, 4:8, :],
                )
                if _SIM_NOCC:
                    nc.gpsimd.dma_start(arout[:], arin[:])
                else:
                    nc.gpsimd.collective_compute(
                        "AllReduce",
                        mybir.AluOpType.add,
                        replica_groups=[list(range(NCORES))],
                        ins=[arin[:]],
                        outs=[arout[:]],
                    )
                htmp = scratch.tile([128, 8, B], bf16, tag="s", name=f"htmp{rep}")
                nc.sync.dma_start(
                    htmp[:, 0:4, :],
                    arout[ds(0, 512), :].rearrange("(m p) d -> p m d", p=128),
                )
                nc.scalar.dma_start(
                    htmp[:, 4:8, :],
                    arout[ds(512, 512), :].rearrange("(m p) d -> p m d", p=128),
                )
                h1 = scratch.tile([128, 8, B], bf16, tag="s", name=f"h1_{rep}")
                for m in range(8):
                    nc.scalar.activation(
                        h1[:, m, :], htmp[:, m, :], RELU, bias=btile[:, m : m + 1]
                    )

                # ---- L2..L5 (one DMA per layer, weights via shared pool) ----
                def mlp_layer(w_dram, k_tiles, m_tiles, rhs, b_idx, func, name, pool,
                              out_dtype=bf16):
                    o = pool.tile(
                        [128, m_tiles, B], out_dtype,
                        tag="s" if pool is scratch else name, name=f"{name}{rep}",
                    )
                    ps = [
                        psum.tile([128, B], f32, tag="psum", name=f"ps{rep}_{name}_{m}")
                        for m in range(m_tiles)
                    ]
                    for kc in range(0, k_tiles, 2):
                        kw = min(2, k_tiles - kc)
                        wt = wpool.tile(
                            [128, kw, m_tiles * 128], bf16, tag="w",
                            name=f"w{rep}_{name}_{kc}",
                        )
                        eng = nc.sync if (kc // 2) % 2 == 0 else nc.scalar
                        eng.dma_start(
                            wt[:],
                            w_dram[ds(kc * 128, kw * 128), :].rearrange(
                                "(k p) d -> p k d", p=128
                            ),
                        )
                        for ki in range(kw):
                            for m in range(m_tiles):
                                nc.tensor.matmul(
                                    ps[m][:],
                                    wt[:, ki, ts(m, 128)],
                                    rhs[:, kc + ki, :],
                                    start=(kc + ki == 0),
                                    stop=(kc + ki == k_tiles - 1),
                                )
                    ob = boff[b_idx]
                    for m in range(m_tiles):
                        nc.scalar.activation(
                            o[:, m, :], ps[m][:], func,
                            bias=btile[:, ob + m : ob + m + 1],
                        )
                    return o

                h2 = mlp_layer(w2, 8, 4, h1, 2, RELU, "h2", scratch)
                ft = mlp_layer(w3, 4, 2, h2, 3, IDENT, "ft", scratch)
                d1 = mlp_layer(w4, 2, 4, ft, 4, RELU, "d1", scratch)
                return mlp_layer(w5, 4, 8, d1, 5, RELU, "d2", acts, out_dtype=fp8)

            def _big(d2):
                # ---- big projection + folded constraint, 32 segments ----
                # Per 512-col segment and batch tile b: one PSUM bank, two
                # independent 256-col accumulation chains (DoubleRow needs
                # N<=256): bf16 interp matmul (start) + 4 fp8 K=256 matmuls.
                for g in range(NSEG // SGRP):
                    ot = [
                        opool.tile(
                            [128, SGRP, SEGC], bf16, tag=f"ot{b}",
                            name=f"ot{rep}_{g}_{b}",
                        )
                        for b in range(4)
                    ]
                    for si in range(SGRP):
                        s = g * SGRP + si
                        ip = ipool.tile(
                            [KI, 2, SEGC], bf16, name=f"ip{rep}_{s}", tag="ip"
                        )
                        nc.scalar.dma_start(ip[:], ipk[s])
                        w6t = w6pool.tile(
                            [128, 8, SEGC], fp8, name=f"w6t{rep}_{s}", tag="w6"
                        )
                        nc.sync.dma_start(
                            w6t[:, 0:4, :],
                            w6e[ds(0, 512), ts(s, SEGC)].rearrange(
                                "(k p) d -> p k d", p=128
                            ),
                        )
                        nc.scalar.dma_start(
                            w6t[:, 4:8, :],
                            w6e[ds(512, 512), ts(s, SEGC)].rearrange(
                                "(k p) d -> p k d", p=128
                            ),
                        )
                        pso = [
                            psum.tile(
                                [128, SEGC], f32, tag="psum", name=f"pso{rep}_{s}_{b}"
                            )
                            for b in range(4)
                        ]
                        for b in range(4):
                            # one full-width start per PSUM bank: HW zeroes at
                            # bank granularity, so per-half starts would clobber
                            nc.tensor.matmul(
                                pso[b][:],
                                ip[:, 0, ts(b, 128)],
                                ip[:, 1, :],
                                start=True,
                                stop=False,
                            )
                        if _USE_DR:
                            for kp in range(4):
                                for b in range(4):
                                    for nh in range(2):
                                        nc.tensor.matmul(
                                            pso[b][:, ds(nh * 256, 256)],
                                            d2[:, ds(2 * kp, 2), ts(b, 128)],
                                            w6t[:, ds(2 * kp, 2), ds(nh * 256, 256)],
                                            start=False,
                                            stop=(kp == 3),
                                            perf_mode=DR,
                                        )
                        else:
                            for ki in range(8):
                                for b in range(4):
                                    nc.tensor.matmul(
                                        pso[b][:],
                                        d2[:, ki, ts(b, 128)],
                                        w6t[:, ki, :],
                                        start=False,
                                        stop=(ki == 7),
                                    )
                        for b in range(4):
                            nc.vector.tensor_scalar_mul(
                                ot[b][:, si, :], pso[b][:], 1.0 / W6SCALE
                            )
                    for b in range(4):
                        eng = nc.sync if b % 2 == 0 else nc.scalar
                        eng.dma_start(
                            out[ts(b, 128), ds(g * SGRP * SEGC, SGRP * SEGC)].rearrange(
                                "p (a x) -> p a x", a=SGRP
                            ),
                            ot[b][:],
                        )

            if phase == "enc":
                d2 = _enc()
                otx = opool.tile([128, B], bf16, name=f"otx{rep}", tag="ot0")
                nc.vector.tensor_copy(otx[:], d2[:, 0, :])
                nc.sync.dma_start(out[ts(0, 128), ts(0, B)], otx[:])
            elif phase == "big":
                d2 = acts.tile([128, 8, B], fp8, tag="d2", name=f"d2x{rep}")
                nc.vector.memset(d2[:].bitcast(DT.uint32), 1)
                _big(d2)
            else:
                _big(_enc())

        for _rep in range(reps):
            _one_rep(_rep)

    nc.compile()
    return nc


def _host_prep(inputs):
    """Shard + fold. Returns per-core input maps."""
    x = np.ascontiguousarray(inputs["low_res_data"], dtype=np.float32)
    x2d = x.reshape(B, LF)
    xTa = np.ascontiguousarray(x2d.T)
    w6 = np.asarray(inputs["w6"], dtype=np.float32)
    b6 = np.asarray(inputs["b6"], dtype=np.float32)

    # per-output-column scale: 0 on knots, 0.2 in-segment, 1.0 in the tail
    h = np.arange(H)
    colscale = np.where(h % UP == 0, 0.0, np.where(h < (L - 1) * UP, 0.2, 1.0))
    colscale = np.repeat(colscale, F).astype(np.float32)  # (HF,)
    b6_eff = b6 * colscale

    # interp coefficient blocks (shared by all segments except the last);
    # coefficient side carries the W6SCALE fp8 pre-scale.
    fidx = np.arange(F)
    std = np.zeros((KI, SEGC), np.float32)
    last = np.zeros((KI, SEGC), np.float32)
    for h_off in range(UP):
        a = h_off / UP
        cs = 1.0 if h_off == 0 else 0.8 * (1.0 - a)
        ce = 0.0 if h_off == 0 else 0.8 * a
        std[fidx, h_off * F + fidx] = cs * W6SCALE
        last[fidx, h_off * F + fidx] = W6SCALE if h_off == 0 else 0.0
        std[F + fidx, h_off * F + fidx] = ce * W6SCALE

    bpk = np.zeros((128, NBIAS), np.float32)
    off = 0
    for i in (1, 2, 3, 4, 5):
        bv = np.asarray(inputs[f"b{i}"], np.float32)
        m = bv.shape[0] // 128
        bpk[:, off : off + m] = bv.reshape(m, 128).T
        off += m

    w1f = np.asarray(inputs["w1"], np.float32)
    shared = {
        "w2": np.asarray(inputs["w2"], np.float32).astype(bfloat16),
        "w3": np.asarray(inputs["w3"], np.float32).astype(bfloat16),
        "w4": np.asarray(inputs["w4"], np.float32).astype(bfloat16),
        "w5": np.asarray(inputs["w5"], np.float32).astype(bfloat16),
        "bpk": bpk,
    }

    in_maps = []
    for c in range(NCORES):
        j0 = c * COLS
        w6e = np.ascontiguousarray(
            w6[:, j0 : j0 + COLS] * colscale[j0 : j0 + COLS] * W6SCALE
        ).astype(float8_e4m3)
        xTsl = np.ascontiguousarray(
            xTa[c * (LF // NCORES) : (c + 1) * (LF // NCORES)]
        ).astype(bfloat16)
        w1sl = np.ascontiguousarray(
            w1f[c * (LF // NCORES) : (c + 1) * (LF // NCORES)]
        ).astype(bfloat16)
        ipk = np.zeros((NSEG, KI, 2, SEGC), np.float32)
        for sl in range(NSEG):
            s = c * NSEG + sl
            ipk[sl, 0:F, 0] = xTa[s * F : (s + 1) * F]
            if s + 1 < L:
                ipk[sl, F : 2 * F, 0] = xTa[(s + 1) * F : (s + 2) * F]
            ipk[sl, 2 * F, 0] = 1.0
            ipk[sl, :, 1] = std if s < L - 1 else last
            ipk[sl, 2 * F, 1] = b6_eff[s * SEGC : (s + 1) * SEGC] * W6SCALE
        in_maps.append(
            {
                **shared,
                "w6e": w6e,
                "ipk": ipk.astype(bfloat16),
                "xTs": xTsl,
                "w1s": w1sl,
            }
        )
    return in_maps


def kernel(**inputs):
    if "nc" not in _CACHE:
        _CACHE["nc"] = _build_program()
    nc = _CACHE["nc"]
    in_maps = _host_prep(inputs)
    res = run_bass_kernel_spmd(nc, in_maps, list(range(NCORES)))
    out = np.empty((B, H, F), np.float32)
    for c in range(NCORES):
        out[:, c * (H // NCORES) : (c + 1) * (H // NCORES), :] = (
            res.results[c]["out"].astype(np.float32).reshape(B, H // NCORES, F)
        )
    # knot timesteps are exact copies of the low-res input; write them in f32
    out[:, ::UP, :] = np.asarray(inputs["low_res_data"], np.float32)
    return out


# revision 35
# speedup vs baseline: 1.0654x; 1.0085x over previous
"""Trainium2 Bass kernel for nn_ConstrainedEnhancementModel.

Model: x(512,256,32) -> flatten(512,8192) -> MLP encoder/decoder
(8192->1024->512->256->512->1024->131072) -> constraint blend with
linearly-interpolated low-res data.

Key transformation (host-side fold): the post-matmul constraint
   out = where(knot, x_knot, where(in_seg, 0.8*interp + 0.2*decoded, decoded))
is linear in x, so it folds into the final projection:
 - scale w6/b6 columns by 0 (knot), 0.2 (in-segment) or 1.0 (tail)
 - per 512-column segment, append a K=65 matmul block:
   32 rows of x (segment start vals), 32 rows (segment end vals), and a
   ones-row carrying the bias b6_eff.
Device kernel is then a pure matmul pipeline (no gather/select on chip).

Precision plan (rel-err budget 2e-2): the dominant projection runs in
fp8 e4m3 with MatmulPerfMode.DoubleRow (K=256/instr, 2x PE rate).  w6 is
pre-scaled by 32 so its N(0, 1/32) entries use e4m3's normal range; the
interp coefficient block is pre-scaled by 32 in bf16, and the PSUM is
descaled by exactly 1/32 during evacuation.  decoded carries only 0.2
weight in most of the output, so its ~4% fp8 error contributes ~1%.
Encoder runs in bf16 end-to-end (including the AllReduce payload).
Output is written bf16 and upcast on host; knot columns are overwritten
host-side with exact f32 input values.

Sharding over 8 cores: encoder L1 is K-sharded + AllReduce at full
batch in transposed layout [features, batch]; L2-L5 replicated; the
1024x131072 projection is tensor-parallel column-sharded (16384 cols =
512 output timesteps per core); core c writes output rows [c*512,
(c+1)*512) of H.

Perf notes: DMA instruction issue costs ~0.6-1us of sequencer time, so
everything is loaded with few, large multi-dim-AP DMAs split across the
two HWDGE-capable engines (sync, scalar).
"""

from contextlib import ExitStack

import numpy as np
from ml_dtypes import bfloat16, float8_e4m3

import concourse.bacc as bacc
import concourse.mybir as mybir
import concourse.tile as tile
from concourse.bass import ds, ts
from concourse.bass_utils import run_bass_kernel_spmd

DT = mybir.dt

B, L, F, H, HID = 512, 256, 32, 4096, 512
UP = H // L          # 16 timesteps per low-res segment
LF = L * F           # 8192
HF = H * F           # 131072
NCORES = 8
COLS = HF // NCORES  # 16384 output cols per core
SEGC = UP * F        # 512 cols per segment
NSEG = COLS // SEGC  # 32 segments per core
KI = 2 * F + 1       # 65: interp-block contraction size
NBIAS = 8 + 4 + 2 + 4 + 8  # packed bias columns
W6SCALE = 32.0       # fp8 pre-scale on w6 / interp coeffs; descaled at PSUM

_CACHE: dict = {}
_SIM_NOCC = False  # sim-only: replace the collective with a local DMA copy
_USE_DR = True     # debug: False = plain fp8 matmuls (K=128) instead of DoubleRow


def _build_program(reps=1, phase="all"):
    """One SPMD program; per-core data differences live in the inputs.

    reps>1 repeats the whole body back-to-back inside one NEFF — used only
    by the timing harness (per-exec time = wall-time delta / extra reps).
    phase: "all" | "enc" (encoder only) | "big" (projection only, dummy d2).
    """
    bf16, f32, fp8 = DT.bfloat16, DT.float32, DT.float8e4
    DR = mybir.MatmulPerfMode.DoubleRow
    nc = bacc.Bacc("TRN2", target_bir_lowering=False, debug=False, num_devices=NCORES)

    KSH = LF // NCORES  # 1024 contraction rows of layer 1 per core
    xTs = nc.dram_tensor("xTs", [KSH, B], bf16, kind="ExternalInput")
    w1s = nc.dram_tensor("w1s", [KSH, 2 * HID], bf16, kind="ExternalInput")
    arin = nc.dram_tensor("arin", [2 * HID, B], bf16)
    arout = nc.dram_tensor("arout", [2 * HID, B], bf16, addr_space="Shared")
    w2 = nc.dram_tensor("w2", [2 * HID, HID], bf16, kind="ExternalInput")
    w3 = nc.dram_tensor("w3", [HID, HID // 2], bf16, kind="ExternalInput")
    w4 = nc.dram_tensor("w4", [HID // 2, HID], bf16, kind="ExternalInput")
    w5 = nc.dram_tensor("w5", [HID, 2 * HID], bf16, kind="ExternalInput")
    bpk = nc.dram_tensor("bpk", [128, NBIAS], f32, kind="ExternalInput")
    w6e = nc.dram_tensor("w6e", [2 * HID, COLS], fp8, kind="ExternalInput")
    ipk = nc.dram_tensor("ipk", [NSEG, KI, 2, SEGC], bf16, kind="ExternalInput")
    out = nc.dram_tensor("out", [B, COLS], bf16, kind="ExternalOutput")

    RELU = mybir.ActivationFunctionType.Relu
    IDENT = mybir.ActivationFunctionType.Identity

    SGRP = 2  # segments per output-staging group

    with tile.TileContext(nc) as tc:

        def _one_rep(rep, ctx):
            psum = ctx.enter_context(
                tc.tile_pool(name=f"psum{rep}", bufs=8, space="PSUM")
            )
            xpool = ctx.enter_context(tc.tile_pool(name=f"xpool{rep}", bufs=4))
            wpool = ctx.enter_context(tc.tile_pool(name=f"wpool{rep}", bufs=5))
            scratch = ctx.enter_context(tc.tile_pool(name=f"scratch{rep}", bufs=2))
            acts = ctx.enter_context(tc.tile_pool(name=f"acts{rep}", bufs=1))
            bpool = ctx.enter_context(tc.tile_pool(name=f"bpool{rep}", bufs=1))
            w6pool = ctx.enter_context(tc.tile_pool(name=f"w6pool{rep}", bufs=3))
            ipool = ctx.enter_context(tc.tile_pool(name=f"ipool{rep}", bufs=2))
            opool = ctx.enter_context(tc.tile_pool(name=f"opool{rep}", bufs=2))

            btile = bpool.tile([128, NBIAS], f32, name="btile")
            nc.scalar.dma_start(btile[:], bpk[:])
            boff = {1: 0, 2: 8, 3: 12, 4: 14, 5: 18}

            def _enc():
                # ---- L1: K-sharded partial matmul + AllReduce over 8 cores ----
                ps1 = [
                    psum.tile([128, B], f32, tag="psum", name=f"ps1_{m}")
                    for m in range(8)
                ]
                for kc in range(4):
                    e1 = nc.sync if kc % 2 == 0 else nc.scalar
                    e2 = nc.scalar if kc % 2 == 0 else nc.sync
                    xt = xpool.tile([128, 2, B], bf16, name=f"xt{kc}", tag="xt")
                    e2.dma_start(
                        xt[:],
                        xTs[ds(kc * 256, 256), :].rearrange("(k p) d -> p k d", p=128),
                    )
                    w1t = wpool.tile([128, 2, 2 * HID], bf16, name=f"w1t{kc}", tag="w")
                    e1.dma_start(
                        w1t[:],
                        w1s[ds(kc * 256, 256), :].rearrange("(k p) d -> p k d", p=128),
                    )
                    for k4 in range(2):
                        for m in range(8):
                            nc.tensor.matmul(
                                ps1[m][:],
                                w1t[:, k4, ts(m, 128)],
                                xt[:, k4, :],
                                start=(kc == 0 and k4 == 0),
                                stop=(kc == 3 and k4 == 1),
                            )
                hp = scratch.tile([128, 8, B], bf16, tag="s", name="hp")
                for m in range(8):
                    nc.vector.tensor_copy(hp[:, m, :], ps1[m][:])
                nc.sync.dma_start(
                    arin[ds(0, 512), :].rearrange("(m p) d -> p m d", p=128),
                    hp[:, 0:4, :],
                )
                nc.scalar.dma_start(
                    arin[ds(512, 512), :].rearrange("(m p) d -> p m d", p=128),
                    hp[:, 4:8, :],
                )
                if _SIM_NOCC:
                    nc.gpsimd.dma_start(arout[:], arin[:])
                else:
                    nc.gpsimd.collective_compute(
                        "AllReduce",
                        mybir.AluOpType.add,
                        replica_groups=[list(range(NCORES))],
                        ins=[arin[:]],
                        outs=[arout[:]],
                    )
                htmp = scratch.tile([128, 8, B], bf16, tag="s", name="htmp")
                nc.sync.dma_start(
                    htmp[:, 0:4, :],
                    arout[ds(0, 512), :].rearrange("(m p) d -> p m d", p=128),
                )
                nc.scalar.dma_start(
                    htmp[:, 4:8, :],
                    arout[ds(512, 512), :].rearrange("(m p) d -> p m d", p=128),
                )
                h1 = scratch.tile([128, 8, B], bf16, tag="s", name="h1")
                for m in range(8):
                    nc.scalar.activation(
                        h1[:, m, :], htmp[:, m, :], RELU, bias=btile[:, m : m + 1]
                    )

                # ---- L2..L5 (one DMA per layer, weights via shared pool) ----
                def mlp_layer(w_dram, k_tiles, m_tiles, rhs, b_idx, func, name, pool,
                              out_dtype=bf16):
                    o = pool.tile(
                        [128, m_tiles, B], out_dtype,
                        tag="s" if pool is scratch else name, name=name,
                    )
                    ps = [
                        psum.tile([128, B], f32, tag="psum", name=f"ps_{name}_{m}")
                        for m in range(m_tiles)
                    ]
                    for kc in range(0, k_tiles, 2):
                        kw = min(2, k_tiles - kc)
                        wt = wpool.tile(
                            [128, kw, m_tiles * 128], bf16, tag="w",
                            name=f"w_{name}_{kc}",
                        )
                        eng = nc.sync if (kc // 2) % 2 == 0 else nc.scalar
                        eng.dma_start(
                            wt[:],
                            w_dram[ds(kc * 128, kw * 128), :].rearrange(
                                "(k p) d -> p k d", p=128
                            ),
                        )
                        for ki in range(kw):
                            for m in range(m_tiles):
                                nc.tensor.matmul(
                                    ps[m][:],
                                    wt[:, ki, ts(m, 128)],
                                    rhs[:, kc + ki, :],
                                    start=(kc + ki == 0),
                                    stop=(kc + ki == k_tiles - 1),
                                )
                    ob = boff[b_idx]
                    for m in range(m_tiles):
                        nc.scalar.activation(
                            o[:, m, :], ps[m][:], func,
                            bias=btile[:, ob + m : ob + m + 1],
                        )
                    return o

                h2 = mlp_layer(w2, 8, 4, h1, 2, RELU, "h2", scratch)
                ft = mlp_layer(w3, 4, 2, h2, 3, IDENT, "ft", scratch)
                d1 = mlp_layer(w4, 2, 4, ft, 4, RELU, "d1", scratch)
                return mlp_layer(w5, 4, 8, d1, 5, RELU, "d2", acts, out_dtype=fp8)

            def _big(d2):
                # ---- big projection + folded constraint, 32 segments ----
                # Per 512-col segment and batch tile b: one PSUM bank, two
                # independent 256-col accumulation chains (DoubleRow needs
                # N<=256): bf16 interp matmul (start) + 4 fp8 K=256 matmuls.
                for g in range(NSEG // SGRP):
                    ot = [
                        opool.tile(
                            [128, SGRP, SEGC], bf16, tag=f"ot{b}", name=f"ot{g}_{b}"
                        )
                        for b in range(4)
                    ]
                    for si in range(SGRP):
                        s = g * SGRP + si
                        ip = ipool.tile([KI, 2, SEGC], bf16, name=f"ip{s}", tag="ip")
                        nc.scalar.dma_start(ip[:], ipk[s])
                        w6t = w6pool.tile(
                            [128, 8, SEGC], fp8, name=f"w6t{s}", tag="w6"
                        )
                        nc.sync.dma_start(
                            w6t[:, 0:4, :],
                            w6e[ds(0, 512), ts(s, SEGC)].rearrange(
                                "(k p) d -> p k d", p=128
                            ),
                        )
                        nc.scalar.dma_start(
                            w6t[:, 4:8, :],
                            w6e[ds(512, 512), ts(s, SEGC)].rearrange(
                                "(k p) d -> p k d", p=128
                            ),
                        )
                        pso = [
                            psum.tile([128, SEGC], f32, tag="psum", name=f"pso_{s}_{b}")
                            for b in range(4)
                        ]
                        for b in range(4):
                            # one full-width start per PSUM bank: HW zeroes at
                            # bank granularity, so per-half starts would clobber
                            nc.tensor.matmul(
                                pso[b][:],
                                ip[:, 0, ts(b, 128)],
                                ip[:, 1, :],
                                start=True,
                                stop=False,
                            )
                        if _USE_DR:
                            for kp in range(4):
                                for b in range(4):
                                    for nh in range(2):
                                        nc.tensor.matmul(
                                            pso[b][:, ds(nh * 256, 256)],
                                            d2[:, ds(2 * kp, 2), ts(b, 128)],
                                            w6t[:, ds(2 * kp, 2), ds(nh * 256, 256)],
                                            start=False,
                                            stop=(kp == 3),
                                            perf_mode=DR,
                                        )
                        else:
                            for ki in range(8):
                                for b in range(4):
                                    nc.tensor.matmul(
                                        pso[b][:],
                                        d2[:, ki, ts(b, 128)],
                                        w6t[:, ki, :],
                                        start=False,
                                        stop=(ki == 7),
                                    )
                        for b in range(4):
                            nc.vector.tensor_scalar_mul(
                                ot[b][:, si, :], pso[b][:], 1.0 / W6SCALE
                            )
                    for b in range(4):
                        eng = nc.sync if b % 2 == 0 else nc.scalar
                        eng.dma_start(
                            out[ts(b, 128), ds(g * SGRP * SEGC, SGRP * SEGC)].rearrange(
                                "p (a x) -> p a x", a=SGRP
                            ),
                            ot[b][:],
                        )

            if phase == "enc":
                d2 = _enc()
                otx = opool.tile([128, B], bf16, name="otx", tag="ot0")
                nc.vector.tensor_copy(otx[:], d2[:, 0, :])
                nc.sync.dma_start(out[ts(0, 128), ts(0, B)], otx[:])
            elif phase == "big":
                d2 = acts.tile([128, 8, B], fp8, tag="d2", name="d2")
                nc.vector.memset(d2[:].bitcast(DT.uint32), 1)
                _big(d2)
            else:
                _big(_enc())

        for _rep in range(reps):
            with ExitStack() as _ctx:
                _one_rep(_rep, _ctx)

    nc.compile()
    return nc


def _host_prep(inputs):
    """Shard + fold. Returns per-core input maps."""
    x = np.ascontiguousarray(inputs["low_res_data"], dtype=np.float32)
    x2d = x.reshape(B, LF)
    xTa = np.ascontiguousarray(x2d.T)
    w6 = np.asarray(inputs["w6"], dtype=np.float32)
    b6 = np.asarray(inputs["b6"], dtype=np.float32)

    # per-output-column scale: 0 on knots, 0.2 in-segment, 1.0 in the tail
    h = np.arange(H)
    colscale = np.where(h % UP == 0, 0.0, np.where(h < (L - 1) * UP, 0.2, 1.0))
    colscale = np.repeat(colscale, F).astype(np.float32)  # (HF,)
    b6_eff = b6 * colscale

    # interp coefficient blocks (shared by all segments except the last);
    # coefficient side carries the W6SCALE fp8 pre-scale.
    fidx = np.arange(F)
    std = np.zeros((KI, SEGC), np.float32)
    last = np.zeros((KI, SEGC), np.float32)
    for h_off in range(UP):
        a = h_off / UP
        cs = 1.0 if h_off == 0 else 0.8 * (1.0 - a)
        ce = 0.0 if h_off == 0 else 0.8 * a
        std[fidx, h_off * F + fidx] = cs * W6SCALE
        last[fidx, h_off * F + fidx] = W6SCALE if h_off == 0 else 0.0
        std[F + fidx, h_off * F + fidx] = ce * W6SCALE

    bpk = np.zeros((128, NBIAS), np.float32)
    off = 0
    for i in (1, 2, 3, 4, 5):
        bv = np.asarray(inputs[f"b{i}"], np.float32)
        m = bv.shape[0] // 128
        bpk[:, off : off + m] = bv.reshape(m, 128).T
        off += m

    w1f = np.asarray(inputs["w1"], np.float32)
    shared = {
        "w2": np.asarray(inputs["w2"], np.float32).astype(bfloat16),
        "w3": np.asarray(inputs["w3"], np.float32).astype(bfloat16),
        "w4": np.asarray(inputs["w4"], np.float32).astype(bfloat16),
        "w5": np.asarray(inputs["w5"], np.float32).astype(bfloat16),
        "bpk": bpk,
    }

    in_maps = []
    for c in range(NCORES):
        j0 = c * COLS
        w6e = np.ascontiguousarray(
            w6[:, j0 : j0 + COLS] * colscale[j0 : j0 + COLS] * W6SCALE
        ).astype(float8_e4m3)
        xTsl = np.ascontiguousarray(
            xTa[c * (LF // NCORES) : (c + 1) * (LF // NCORES)]
        ).astype(bfloat16)
        w1sl = np.ascontiguousarray(
            w1f[c * (LF // NCORES) : (c + 1) * (LF // NCORES)]
        ).astype(bfloat16)
        ipk = np.zeros((NSEG, KI, 2, SEGC), np.float32)
        for sl in range(NSEG):
            s = c * NSEG + sl
            ipk[sl, 0:F, 0] = xTa[s * F : (s + 1) * F]
            if s + 1 < L:
                ipk[sl, F : 2 * F, 0] = xTa[(s + 1) * F : (s + 2) * F]
            ipk[sl, 2 * F, 0] = 1.0
            ipk[sl, :, 1] = std if s < L - 1 else last
            ipk[sl, 2 * F, 1] = b6_eff[s * SEGC : (s + 1) * SEGC] * W6SCALE
        in_maps.append(
            {
                **shared,
                "w6e": w6e,
                "ipk": ipk.astype(bfloat16),
                "xTs": xTsl,
                "w1s": w1sl,
            }
        )
    return in_maps


def kernel(**inputs):
    if "nc" not in _CACHE:
        _CACHE["nc"] = _build_program()
    nc = _CACHE["nc"]
    in_maps = _host_prep(inputs)
    res = run_bass_kernel_spmd(nc, in_maps, list(range(NCORES)))
    out = np.empty((B, H, F), np.float32)
    for c in range(NCORES):
        out[:, c * (H // NCORES) : (c + 1) * (H // NCORES), :] = (
            res.results[c]["out"].astype(np.float32).reshape(B, H // NCORES, F)
        )
    # knot timesteps are exact copies of the low-res input; write them in f32
    out[:, ::UP, :] = np.asarray(inputs["low_res_data"], np.float32)
    return out


# revision 43
# speedup vs baseline: 1.1701x; 1.0983x over previous
"""Trainium2 Bass kernel for nn_ConstrainedEnhancementModel.

Model: x(512,256,32) -> flatten(512,8192) -> MLP encoder/decoder
(8192->1024->512->256->512->1024->131072) -> constraint blend with
linearly-interpolated low-res data.

Key transformation (host-side fold): the post-matmul constraint
   out = where(knot, x_knot, where(in_seg, 0.8*interp + 0.2*decoded, decoded))
is linear in x, so it folds into the final projection:
 - scale w6/b6 columns by 0 (knot), 0.2 (in-segment) or 1.0 (tail)
 - per 512-column segment, append a K=65 matmul block:
   32 rows of x (segment start vals), 32 rows (segment end vals), and a
   ones-row carrying the bias b6_eff.
Device kernel is then a pure matmul pipeline (no gather/select on chip).

Precision plan (rel-err budget 2e-2): the dominant projection runs in
fp8 e4m3 with MatmulPerfMode.DoubleRow (K=256/instr, 2x PE rate).  w6 is
pre-scaled by 32 so its N(0, 1/32) entries use e4m3's normal range; the
interp coefficient block is pre-scaled by 32 in bf16, and the PSUM is
descaled by exactly 1/32 during evacuation.  decoded carries only 0.2
weight in most of the output, so its ~4% fp8 error contributes ~1%.
Encoder runs in bf16 end-to-end (including the AllReduce payload).
Output is written bf16 and upcast on host; knot columns are overwritten
host-side with exact f32 input values.

Sharding over 8 cores: encoder L1 is K-sharded + AllReduce at full
batch in transposed layout [features, batch]; L2-L5 replicated; the
1024x131072 projection is tensor-parallel column-sharded (16384 cols =
512 output timesteps per core); core c writes output rows [c*512,
(c+1)*512) of H.

Perf notes: DMA instruction issue costs ~0.6-1us of sequencer time, so
everything is loaded with few, large multi-dim-AP DMAs split across the
two HWDGE-capable engines (sync, scalar).
"""

from contextlib import ExitStack

import numpy as np
from ml_dtypes import bfloat16, float8_e4m3

import concourse.bacc as bacc
import concourse.mybir as mybir
import concourse.tile as tile
from concourse.bass import ds, ts
from concourse.bass_utils import run_bass_kernel_spmd

DT = mybir.dt

B, L, F, H, HID = 512, 256, 32, 4096, 512
UP = H // L          # 16 timesteps per low-res segment
LF = L * F           # 8192
HF = H * F           # 131072
NCORES = 8
COLS = HF // NCORES  # 16384 output cols per core
SEGC = UP * F        # 512 cols per segment
NSEG = COLS // SEGC  # 32 segments per core
KI = 2 * F + 1       # 65: interp-block contraction size
NBIAS = 8 + 4 + 2 + 4 + 8  # packed bias columns
W6SCALE = 32.0       # fp8 pre-scale on w6 / interp coeffs; descaled at PSUM

_CACHE: dict = {}
_SIM_NOCC = False  # sim-only: replace the collective with a local DMA copy
_USE_DR = True     # debug: False = plain fp8 matmuls (K=128) instead of DoubleRow


def _build_program(reps=1, phase="all"):
    """One SPMD program; per-core data differences live in the inputs.

    reps>1 repeats the whole body back-to-back inside one NEFF — used only
    by the timing harness (per-exec time = wall-time delta / extra reps).
    phase: "all" | "enc" (encoder only) | "big" (projection only, dummy d2).
    """
    bf16, f32, fp8 = DT.bfloat16, DT.float32, DT.float8e4
    DR = mybir.MatmulPerfMode.DoubleRow
    nc = bacc.Bacc("TRN2", target_bir_lowering=False, debug=False, num_devices=NCORES)

    KSH = LF // NCORES  # 1024 contraction rows of layer 1 per core
    xTs = nc.dram_tensor("xTs", [KSH, B], bf16, kind="ExternalInput")
    w1s = nc.dram_tensor("w1s", [KSH, 2 * HID], bf16, kind="ExternalInput")
    arin = nc.dram_tensor("arin", [2 * HID, B], bf16)
    arout = nc.dram_tensor("arout", [2 * HID, B], bf16, addr_space="Shared")
    w2 = nc.dram_tensor("w2", [2 * HID, HID], bf16, kind="ExternalInput")
    w3 = nc.dram_tensor("w3", [HID, HID // 2], bf16, kind="ExternalInput")
    w4 = nc.dram_tensor("w4", [HID // 2, HID], bf16, kind="ExternalInput")
    w5 = nc.dram_tensor("w5", [HID, 2 * HID], bf16, kind="ExternalInput")
    bpk = nc.dram_tensor("bpk", [128, NBIAS], f32, kind="ExternalInput")
    w6e = nc.dram_tensor("w6e", [2 * HID, COLS], fp8, kind="ExternalInput")
    ipk = nc.dram_tensor("ipk", [NSEG, KI, 2, SEGC], bf16, kind="ExternalInput")
    out = nc.dram_tensor("out", [B, COLS], bf16, kind="ExternalOutput")

    RELU = mybir.ActivationFunctionType.Relu
    IDENT = mybir.ActivationFunctionType.Identity

    SGRP = 2  # segments per output-staging group
    PSTG = 10  # segments whose interp matmul is pre-staged during the AllReduce

    with tile.TileContext(nc) as tc:

        def _one_rep(rep, ctx):
            psum = ctx.enter_context(
                tc.tile_pool(name=f"psum{rep}", bufs=8, space="PSUM")
            )
            xpool = ctx.enter_context(tc.tile_pool(name=f"xpool{rep}", bufs=4))
            wpool = ctx.enter_context(tc.tile_pool(name=f"wpool{rep}", bufs=5))
            scratch = ctx.enter_context(tc.tile_pool(name=f"scratch{rep}", bufs=2))
            acts = ctx.enter_context(tc.tile_pool(name=f"acts{rep}", bufs=1))
            bpool = ctx.enter_context(tc.tile_pool(name=f"bpool{rep}", bufs=1))
            w6pool = ctx.enter_context(tc.tile_pool(name=f"w6pool{rep}", bufs=3))
            ipool = ctx.enter_context(tc.tile_pool(name=f"ipool{rep}", bufs=2))
            opool = ctx.enter_context(tc.tile_pool(name=f"opool{rep}", bufs=2))
            stpool = ctx.enter_context(tc.tile_pool(name=f"stpool{rep}", bufs=1))

            btile = bpool.tile([128, NBIAS], f32, name="btile")
            nc.scalar.dma_start(btile[:], bpk[:])
            boff = {1: 0, 2: 8, 3: 12, 4: 14, 5: 18}

            def _enc():
                # ---- L1: K-sharded partial matmul + AllReduce over 8 cores ----
                ps1 = [
                    psum.tile([128, B], f32, tag="psum", name=f"ps1_{m}")
                    for m in range(8)
                ]
                for kc in range(4):
                    e1 = nc.sync if kc % 2 == 0 else nc.scalar
                    e2 = nc.scalar if kc % 2 == 0 else nc.sync
                    xt = xpool.tile([128, 2, B], bf16, name=f"xt{kc}", tag="xt")
                    e2.dma_start(
                        xt[:],
                        xTs[ds(kc * 256, 256), :].rearrange("(k p) d -> p k d", p=128),
                    )
                    w1t = wpool.tile([128, 2, 2 * HID], bf16, name=f"w1t{kc}", tag="w")
                    e1.dma_start(
                        w1t[:],
                        w1s[ds(kc * 256, 256), :].rearrange("(k p) d -> p k d", p=128),
                    )
                    for k4 in range(2):
                        for m in range(8):
                            nc.tensor.matmul(
                                ps1[m][:],
                                w1t[:, k4, ts(m, 128)],
                                xt[:, k4, :],
                                start=(kc == 0 and k4 == 0),
                                stop=(kc == 3 and k4 == 1),
                            )
                hp = scratch.tile([128, 8, B], bf16, tag="s", name="hp")
                for m in range(8):
                    nc.vector.tensor_copy(hp[:, m, :], ps1[m][:])
                nc.sync.dma_start(
                    arin[ds(0, 512), :].rearrange("(m p) d -> p m d", p=128),
                    hp[:, 0:4, :],
                )
                nc.scalar.dma_start(
                    arin[ds(512, 512), :].rearrange("(m p) d -> p m d", p=128),
                    hp[:, 4:8, :],
                )
                if _SIM_NOCC:
                    nc.gpsimd.dma_start(arout[:], arin[:])
                else:
                    nc.gpsimd.collective_compute(
                        "AllReduce",
                        mybir.AluOpType.add,
                        replica_groups=[list(range(NCORES))],
                        ins=[arin[:]],
                        outs=[arout[:]],
                    )
                # ---- fill the AllReduce wait (~60us measured end-to-end, PE
                # otherwise idle + HAM-cold) with the first PSTG segments'
                # interp matmuls, staged to SBUF.  Their big-phase chains then
                # start on the first fp8 matmul and re-add ibase at evacuation.
                ibase = []
                for s in range(PSTG):
                    ipx = ipool.tile([KI, 2, SEGC], bf16, name=f"ipx{s}", tag="ip")
                    nc.scalar.dma_start(ipx[:], ipk[s])
                    ib = stpool.tile(
                        [128, 4, SEGC], bf16, name=f"ib{s}", tag=f"ib{s}"
                    )
                    psi = [
                        psum.tile([128, SEGC], f32, tag="psum", name=f"psi_{s}_{b}")
                        for b in range(4)
                    ]
                    for b in range(4):
                        nc.tensor.matmul(
                            psi[b][:],
                            ipx[:, 0, ts(b, 128)],
                            ipx[:, 1, :],
                            start=True,
                            stop=True,
                        )
                    for b in range(4):
                        nc.vector.tensor_scalar_mul(
                            ib[:, b, :], psi[b][:], 1.0 / W6SCALE
                        )
                    ibase.append(ib)

                htmp = scratch.tile([128, 8, B], bf16, tag="s", name="htmp")
                nc.sync.dma_start(
                    htmp[:, 0:4, :],
                    arout[ds(0, 512), :].rearrange("(m p) d -> p m d", p=128),
                )
                nc.scalar.dma_start(
                    htmp[:, 4:8, :],
                    arout[ds(512, 512), :].rearrange("(m p) d -> p m d", p=128),
                )
                h1 = scratch.tile([128, 8, B], bf16, tag="s", name="h1")
                for m in range(8):
                    nc.scalar.activation(
                        h1[:, m, :], htmp[:, m, :], RELU, bias=btile[:, m : m + 1]
                    )

                # ---- L2..L5 (one DMA per layer, weights via shared pool) ----
                def mlp_layer(w_dram, k_tiles, m_tiles, rhs, b_idx, func, name, pool,
                              out_dtype=bf16):
                    o = pool.tile(
                        [128, m_tiles, B], out_dtype,
                        tag="s" if pool is scratch else name, name=name,
                    )
                    ps = [
                        psum.tile([128, B], f32, tag="psum", name=f"ps_{name}_{m}")
                        for m in range(m_tiles)
                    ]
                    for kc in range(0, k_tiles, 2):
                        kw = min(2, k_tiles - kc)
                        wt = wpool.tile(
                            [128, kw, m_tiles * 128], bf16, tag="w",
                            name=f"w_{name}_{kc}",
                        )
                        eng = nc.sync if (kc // 2) % 2 == 0 else nc.scalar
                        eng.dma_start(
                            wt[:],
                            w_dram[ds(kc * 128, kw * 128), :].rearrange(
                                "(k p) d -> p k d", p=128
                            ),
                        )
                        for ki in range(kw):
                            for m in range(m_tiles):
                                nc.tensor.matmul(
                                    ps[m][:],
                                    wt[:, ki, ts(m, 128)],
                                    rhs[:, kc + ki, :],
                                    start=(kc + ki == 0),
                                    stop=(kc + ki == k_tiles - 1),
                                )
                    ob = boff[b_idx]
                    for m in range(m_tiles):
                        nc.scalar.activation(
                            o[:, m, :], ps[m][:], func,
                            bias=btile[:, ob + m : ob + m + 1],
                        )
                    return o

                h2 = mlp_layer(w2, 8, 4, h1, 2, RELU, "h2", scratch)
                ft = mlp_layer(w3, 4, 2, h2, 3, IDENT, "ft", scratch)
                d1 = mlp_layer(w4, 2, 4, ft, 4, RELU, "d1", scratch)
                d2 = mlp_layer(w5, 4, 8, d1, 5, RELU, "d2", acts, out_dtype=fp8)
                return d2, ibase

            def _big(d2, ibase):
                # ---- big projection + folded constraint, 32 segments ----
                # Per 512-col segment and batch tile b: one PSUM bank, two
                # independent 256-col accumulation chains (DoubleRow needs
                # N<=256): bf16 interp matmul (start) + 4 fp8 K=256 matmuls.
                for g in range(NSEG // SGRP):
                    ot = [
                        opool.tile(
                            [128, SGRP, SEGC], bf16, tag=f"ot{b}", name=f"ot{g}_{b}"
                        )
                        for b in range(4)
                    ]
                    for si in range(SGRP):
                        s = g * SGRP + si
                        staged = s < len(ibase)
                        if not staged:
                            ip = ipool.tile(
                                [KI, 2, SEGC], bf16, name=f"ip{s}", tag="ip"
                            )
                            nc.scalar.dma_start(ip[:], ipk[s])
                        w6t = w6pool.tile(
                            [128, 8, SEGC], fp8, name=f"w6t{s}", tag="w6"
                        )
                        nc.sync.dma_start(
                            w6t[:, 0:4, :],
                            w6e[ds(0, 512), ts(s, SEGC)].rearrange(
                                "(k p) d -> p k d", p=128
                            ),
                        )
                        nc.scalar.dma_start(
                            w6t[:, 4:8, :],
                            w6e[ds(512, 512), ts(s, SEGC)].rearrange(
                                "(k p) d -> p k d", p=128
                            ),
                        )
                        pso = [
                            psum.tile([128, SEGC], f32, tag="psum", name=f"pso_{s}_{b}")
                            for b in range(4)
                        ]
                        if staged:
                            # interp already in SBUF: full-width DR chain, the
                            # kp=0 matmul zeroes the bank (full-bank start, so
                            # no per-half clobber); ibase re-added at evac.
                            for kp in range(4):
                                for b in range(4):
                                    nc.tensor.matmul(
                                        pso[b][:],
                                        d2[:, ds(2 * kp, 2), ts(b, 128)],
                                        w6t[:, ds(2 * kp, 2), :],
                                        start=(kp == 0),
                                        stop=(kp == 3),
                                        perf_mode=DR,
                                    )
                            for b in range(4):
                                nc.vector.scalar_tensor_tensor(
                                    ot[b][:, si, :],
                                    pso[b][:],
                                    1.0 / W6SCALE,
                                    ibase[s][:, b, :],
                                    mybir.AluOpType.mult,
                                    mybir.AluOpType.add,
                                )
                            continue
                        for b in range(4):
                            # one full-width start per PSUM bank: HW zeroes at
                            # bank granularity, so per-half starts would clobber
                            nc.tensor.matmul(
                                pso[b][:],
                                ip[:, 0, ts(b, 128)],
                                ip[:, 1, :],
                                start=True,
                                stop=False,
                            )
                        if _USE_DR:
                            for kp in range(4):
                                for b in range(4):
                                    for nh in range(2):
                                        nc.tensor.matmul(
                                            pso[b][:, ds(nh * 256, 256)],
                                            d2[:, ds(2 * kp, 2), ts(b, 128)],
                                            w6t[:, ds(2 * kp, 2), ds(nh * 256, 256)],
                                            start=False,
                                            stop=(kp == 3),
                                            perf_mode=DR,
                                        )
                        else:
                            for ki in range(8):
                                for b in range(4):
                                    nc.tensor.matmul(
                                        pso[b][:],
                                        d2[:, ki, ts(b, 128)],
                                        w6t[:, ki, :],
                                        start=False,
                                        stop=(ki == 7),
                                    )
                        for b in range(4):
                            nc.vector.tensor_scalar_mul(
                                ot[b][:, si, :], pso[b][:], 1.0 / W6SCALE
                            )
                    for b in range(4):
                        eng = nc.sync if b % 2 == 0 else nc.scalar
                        eng.dma_start(
                            out[ts(b, 128), ds(g * SGRP * SEGC, SGRP * SEGC)].rearrange(
                                "p (a x) -> p a x", a=SGRP
                            ),
                            ot[b][:],
                        )

            if phase == "enc":
                d2, _ib = _enc()
                otx = opool.tile([128, B], bf16, name="otx", tag="ot0")
                nc.vector.tensor_copy(otx[:], d2[:, 0, :])
                nc.sync.dma_start(out[ts(0, 128), ts(0, B)], otx[:])
            elif phase == "big":
                d2 = acts.tile([128, 8, B], fp8, tag="d2", name="d2")
                nc.vector.memset(d2[:].bitcast(DT.uint32), 1)
                _big(d2, [])
            else:
                d2, ibase = _enc()
                _big(d2, ibase)

        for _rep in range(reps):
            with ExitStack() as _ctx:
                _one_rep(_rep, _ctx)

    nc.compile()
    return nc


def _host_prep(inputs):
    """Shard + fold. Returns per-core input maps."""
    x = np.ascontiguousarray(inputs["low_res_data"], dtype=np.float32)
    x2d = x.reshape(B, LF)
    xTa = np.ascontiguousarray(x2d.T)
    w6 = np.asarray(inputs["w6"], dtype=np.float32)
    b6 = np.asarray(inputs["b6"], dtype=np.float32)

    # per-output-column scale: 0 on knots, 0.2 in-segment, 1.0 in the tail
    h = np.arange(H)
    colscale = np.where(h % UP == 0, 0.0, np.where(h < (L - 1) * UP, 0.2, 1.0))
    colscale = np.repeat(colscale, F).astype(np.float32)  # (HF,)
    b6_eff = b6 * colscale

    # interp coefficient blocks (shared by all segments except the last);
    # coefficient side carries the W6SCALE fp8 pre-scale.
    fidx = np.arange(F)
    std = np.zeros((KI, SEGC), np.float32)
    last = np.zeros((KI, SEGC), np.float32)
    for h_off in range(UP):
        a = h_off / UP
        cs = 1.0 if h_off == 0 else 0.8 * (1.0 - a)
        ce = 0.0 if h_off == 0 else 0.8 * a
        std[fidx, h_off * F + fidx] = cs * W6SCALE
        last[fidx, h_off * F + fidx] = W6SCALE if h_off == 0 else 0.0
        std[F + fidx, h_off * F + fidx] = ce * W6SCALE

    bpk = np.zeros((128, NBIAS), np.float32)
    off = 0
    for i in (1, 2, 3, 4, 5):
        bv = np.asarray(inputs[f"b{i}"], np.float32)
        m = bv.shape[0] // 128
        bpk[:, off : off + m] = bv.reshape(m, 128).T
        off += m

    w1f = np.asarray(inputs["w1"], np.float32)
    shared = {
        "w2": np.asarray(inputs["w2"], np.float32).astype(bfloat16),
        "w3": np.asarray(inputs["w3"], np.float32).astype(bfloat16),
        "w4": np.asarray(inputs["w4"], np.float32).astype(bfloat16),
        "w5": np.asarray(inputs["w5"], np.float32).astype(bfloat16),
        "bpk": bpk,
    }

    in_maps = []
    for c in range(NCORES):
        j0 = c * COLS
        w6e = np.ascontiguousarray(
            w6[:, j0 : j0 + COLS] * colscale[j0 : j0 + COLS] * W6SCALE
        ).astype(float8_e4m3)
        xTsl = np.ascontiguousarray(
            xTa[c * (LF // NCORES) : (c + 1) * (LF // NCORES)]
        ).astype(bfloat16)
        w1sl = np.ascontiguousarray(
            w1f[c * (LF // NCORES) : (c + 1) * (LF // NCORES)]
        ).astype(bfloat16)
        ipk = np.zeros((NSEG, KI, 2, SEGC), np.float32)
        for sl in range(NSEG):
            s = c * NSEG + sl
            ipk[sl, 0:F, 0] = xTa[s * F : (s + 1) * F]
            if s + 1 < L:
                ipk[sl, F : 2 * F, 0] = xTa[(s + 1) * F : (s + 2) * F]
            ipk[sl, 2 * F, 0] = 1.0
            ipk[sl, :, 1] = std if s < L - 1 else last
            ipk[sl, 2 * F, 1] = b6_eff[s * SEGC : (s + 1) * SEGC] * W6SCALE
        in_maps.append(
            {
                **shared,
                "w6e": w6e,
                "ipk": ipk.astype(bfloat16),
                "xTs": xTsl,
                "w1s": w1sl,
            }
        )
    return in_maps


def kernel(**inputs):
    if "nc" not in _CACHE:
        _CACHE["nc"] = _build_program()
    nc = _CACHE["nc"]
    in_maps = _host_prep(inputs)
    res = run_bass_kernel_spmd(nc, in_maps, list(range(NCORES)))
    out = np.empty((B, H, F), np.float32)
    for c in range(NCORES):
        out[:, c * (H // NCORES) : (c + 1) * (H // NCORES), :] = (
            res.results[c]["out"].astype(np.float32).reshape(B, H // NCORES, F)
        )
    # knot timesteps are exact copies of the low-res input; write them in f32
    out[:, ::UP, :] = np.asarray(inputs["low_res_data"], np.float32)
    return out
